# revision 6
# baseline (speedup 1.0000x reference)
"""Trainium2 Bass kernel for nn_DSSnetwork (DSS-GNN message passing).

Strategy
--------
* 512 subgraphs x 512 nodes; edges never cross subgraphs. Shard 64
  subgraphs per core across 8 cores (graph parallel).
* Aggregation per subgraph becomes a dense matmul with the subgraph's
  512x512 adjacency *count* matrix, densified on host. Counts are small
  integers, exact in bf16, so the adjacency streams from HBM at half
  the fp32 bytes. aggT[e,d] = sum_s h[s,e] * A[s,d] = (h^T A)[e,d].
* Node features live on-chip, transposed: hT [64 chan, 32768 nodes]
  (f32). Per 512-node chunk s the layer does:
    - 4 PE transposes of hT chunk -> h normal [128,64] blocks (cast bf16)
    - 4 accumulating matmuls vs streamed A tiles -> aggT [64,512] (PSUM)
    - 2 matmuls (Wrel^T @ aggT + Wroot^T @ hT_chunk) -> h1pre (PSUM)
    - PSUM->SBUF copy overwrites the hT chunk in place (+ fused per-
      partition sum for BN stats); sum-of-squares via one DVE op.
* BatchNorm batch stats and the subgraph scatter-mean (subgraph_node_idx
  is tile(arange(512),512), so it's a strided mean with count 512) are
  combined via small DRAM AllReduces across the 8 cores.
* brel/brel_s are dropped: a pre-BN bias shifts the batch mean only, so
  BatchNorm cancels it exactly.
* The tiny [512,*] summary branch + final log_softmax/MLP are computed
  replicated on every core; core 0's output is returned.
"""

import numpy as np
import ml_dtypes

import concourse.bass as bass
import concourse.tile as tile
from concourse import bacc, mybir
from concourse.bass_utils import run_bass_kernel_spmd
from concourse.masks import make_identity

F32 = mybir.dt.float32
BF16 = mybir.dt.bfloat16
AF = mybir.ActivationFunctionType
ALU = mybir.AluOpType
AX = mybir.AxisListType

N = 512        # nodes per subgraph == bins of the scatter-mean
L = 4          # layers
EMB = 64
TASKS = 10
KC = N // 128  # 4 k-chunks per subgraph
BF = ml_dtypes.bfloat16

N_CORES = 8
S_LOCAL = 64   # subgraphs per core (full problem)


def build_nc(n_cores=N_CORES, s_local=S_LOCAL, agg_bf16=True):
    nodes = s_local * N           # nodes per core
    s_total = n_cores * s_local
    nt_total = s_total * N
    adt = BF16 if agg_bf16 else F32

    nc = bacc.Bacc(
        "TRN2",
        target_bir_lowering=False,
        debug=False,
        enable_asserts=True,
        num_devices=n_cores,
    )
    # ---- DRAM I/O ----
    a_big = nc.dram_tensor("a_big", [s_local, N, N], adt, kind="ExternalInput").ap()
    h0T = nc.dram_tensor("h0T", [EMB, nodes], F32, kind="ExternalInput").ap()
    a_orig = nc.dram_tensor("a_orig", [N, N], F32, kind="ExternalInput").ap()
    wrelT = nc.dram_tensor("wrelT", [EMB, L, EMB], F32, kind="ExternalInput").ap()
    wrootT = nc.dram_tensor("wrootT", [EMB, L, EMB], F32, kind="ExternalInput").ap()
    wrelsT = nc.dram_tensor("wrelsT", [EMB, L, EMB], F32, kind="ExternalInput").ap()
    wrootsT = nc.dram_tensor("wrootsT", [EMB, L, EMB], F32, kind="ExternalInput").ap()
    # per-channel vectors: columns [bn_g(L) | bn_b(L) | bns_g(L) | bns_b(L)]
    chvecs = nc.dram_tensor("chvecs", [EMB, 4 * L], F32, kind="ExternalInput").ap()
    w1T = nc.dram_tensor("w1T", [EMB, 2 * EMB], F32, kind="ExternalInput").ap()
    b1c = nc.dram_tensor("b1c", [2 * EMB, 1], F32, kind="ExternalInput").ap()
    w2T = nc.dram_tensor("w2T", [2 * EMB, TASKS], F32, kind="ExternalInput").ap()
    b2c = nc.dram_tensor("b2c", [TASKS, 1], F32, kind="ExternalInput").ap()
    outT = nc.dram_tensor("outT", [TASKS, N], F32, kind="ExternalOutput").ap()

    groups = [list(range(n_cores))]
    # Shared scratchpad outputs only supported for >4-core groups
    cc_space = "Shared" if n_cores > 4 else "Local"

    with tile.TileContext(nc) as tc:
        with (
            tc.tile_pool(name="state", bufs=1) as state,
            tc.tile_pool(name="apool", bufs=2) as apool,
            tc.tile_pool(name="hnpool", bufs=3) as hnpool,
            tc.tile_pool(name="aggsb", bufs=3) as aggsb_pool,
            tc.tile_pool(name="scr", bufs=3) as scr_pool,
            tc.tile_pool(name="smol", bufs=4) as smol,
            tc.tile_pool(name="pstp", bufs=2, space="PSUM") as pstp,
            tc.tile_pool(name="psa", bufs=2, space="PSUM") as psa,
            tc.tile_pool(name="psb", bufs=2, space="PSUM") as psb,
            tc.tile_pool(name="dram", bufs=2, space="DRAM") as dram,
        ):
            # ---- persistent state ----
            hbuf = state.tile([EMB, nodes], F32)
            ident = state.tile([128, 128], F32)
            make_identity(nc, ident)
            eps_t = state.tile([EMB, 1], F32)
            nc.vector.memset(eps_t, 1e-5)

            wrel_sb = state.tile([EMB, L, EMB], F32)
            nc.sync.dma_start(out=wrel_sb, in_=wrelT)
            wroot_sb = state.tile([EMB, L, EMB], F32)
            nc.sync.dma_start(out=wroot_sb, in_=wrootT)
            wrels_sb = state.tile([EMB, L, EMB], F32)
            nc.sync.dma_start(out=wrels_sb, in_=wrelsT)
            wroots_sb = state.tile([EMB, L, EMB], F32)
            nc.sync.dma_start(out=wroots_sb, in_=wrootsT)
            chv_sb = state.tile([EMB, 4 * L], F32)
            nc.sync.dma_start(out=chv_sb, in_=chvecs)
            aorig_sb = state.tile([128, KC, N], F32)
            nc.sync.dma_start(
                out=aorig_sb, in_=a_orig.rearrange("(k p) d -> p k d", p=128)
            )
            w1_sb = state.tile([EMB, 2 * EMB], F32)
            nc.sync.dma_start(out=w1_sb, in_=w1T)
            b1_sb = state.tile([2 * EMB, 1], F32)
            nc.sync.dma_start(out=b1_sb, in_=b1c)
            w2_sb = state.tile([2 * EMB, TASKS], F32)
            nc.sync.dma_start(out=w2_sb, in_=w2T)
            b2_sb = state.tile([TASKS, 1], F32)
            nc.sync.dma_start(out=b2_sb, in_=b2c)

            # initial hT load, chunked so layer 1 can start early
            for s in range(s_local):
                nc.sync.dma_start(
                    out=hbuf[:, s * N:(s + 1) * N], in_=h0T[:, s * N:(s + 1) * N]
                )

            def xsum_allreduce(tag):
                """Partial sum of hbuf chunks over local subgraphs, then
                AllReduce; returns SBUF tile [EMB, N] with the global mean
                (i.e. x_sum of the reference)."""
                acc = state.tile([EMB, N], F32, name=f"xsacc_{tag}", tag="xsacc")
                nc.vector.tensor_copy(acc, hbuf[:, 0:N])
                for s in range(1, s_local):
                    nc.vector.tensor_tensor(
                        out=acc, in0=acc, in1=hbuf[:, s * N:(s + 1) * N], op=ALU.add
                    )
                if n_cores > 1:
                    ari = dram.tile([EMB, N], F32, name=f"xsari_{tag}", tag="xsari")
                    aro = dram.tile(
                        [EMB, N], F32, name=f"xsaro_{tag}", tag="xsaro",
                        addr_space=cc_space,
                    )
                    nc.sync.dma_start(out=ari, in_=acc)
                    nc.gpsimd.collective_compute(
                        "AllReduce", ALU.add, replica_groups=groups,
                        ins=[ari.opt()], outs=[aro.opt()],
                    )
                    tot = state.tile([EMB, N], F32, name=f"xstot_{tag}", tag="xstot")
                    nc.sync.dma_start(out=tot, in_=aro)
                else:
                    tot = acc
                mean = state.tile([EMB, N], F32, name=f"xsmean_{tag}", tag="xsmean")
                nc.vector.tensor_scalar_mul(mean, tot, 1.0 / s_total)
                return mean

            def bn_vectors(mu, var, g_col, b_col, tag):
                """-> (sg, bp): y = x*sg + bp applies the BN. [EMB,1] tiles."""
                sd = smol.tile([EMB, 1], F32, name=f"sd_{tag}", tag="sd")
                nc.scalar.activation(out=sd, in_=var, func=AF.Sqrt, bias=eps_t)
                rstd = smol.tile([EMB, 1], F32, name=f"rstd_{tag}", tag="rstd")
                nc.vector.reciprocal(rstd, sd)
                sg = smol.tile([EMB, 1], F32, name=f"sg_{tag}", tag="sg")
                nc.vector.tensor_tensor(out=sg, in0=rstd, in1=g_col, op=ALU.mult)
                bp = smol.tile([EMB, 1], F32, name=f"bp_{tag}", tag="bp")
                nc.vector.scalar_tensor_tensor(
                    out=bp, in0=mu, scalar=-1.0, in1=sg, op0=ALU.mult, op1=ALU.mult
                )
                nc.vector.tensor_tensor(out=bp, in0=bp, in1=b_col, op=ALU.add)
                return sg, bp

            for l in range(L):
                # x_sum of the layer input (consumed by the summary branch)
                xmean = xsum_allreduce(f"l{l}")

                # ---- main branch: stream subgraphs ----
                ssum = state.tile([EMB, s_local], F32, name=f"ssum{l}", tag="ssum")
                ssq = state.tile([EMB, s_local], F32, name=f"ssq{l}", tag="ssq")
                for s in range(s_local):
                    ch = hbuf[:, s * N:(s + 1) * N]
                    # h chunk -> normal layout bf16 blocks [128, KC, 64]
                    tp_ps = pstp.tile([128, KC, EMB], F32, name="tp", tag="tp")
                    for k in range(KC):
                        nc.tensor.transpose(
                            tp_ps[:, k, :],
                            ch[:, k * 128:(k + 1) * 128],
                            ident[0:EMB, 0:EMB],
                        )
                    hn = hnpool.tile([128, KC, EMB], adt, name="hn", tag="hn")
                    nc.scalar.copy(hn, tp_ps)
                    # stream A tiles for this subgraph
                    at = apool.tile([128, KC, N], adt, name="at", tag="at")
                    nc.sync.dma_start(
                        out=at, in_=a_big[s].rearrange("(k p) d -> p k d", p=128)
                    )
                    agg_ps = psa.tile([EMB, N], F32, name="aggps", tag="aggps")
                    for k in range(KC):
                        nc.tensor.matmul(
                            agg_ps, hn[:, k, :], at[:, k, :],
                            start=(k == 0), stop=(k == KC - 1),
                        )
                    agg_sb = aggsb_pool.tile([EMB, N], F32, name="aggsb", tag="aggsb")
                    nc.scalar.copy(agg_sb, agg_ps)
                    h1_ps = psb.tile([EMB, N], F32, name="h1ps", tag="h1ps")
                    nc.tensor.matmul(
                        h1_ps, wrel_sb[:, l, :], agg_sb, start=True, stop=False
                    )
                    nc.tensor.matmul(
                        h1_ps, wroot_sb[:, l, :], ch, start=False, stop=True
                    )
                    # overwrite hT chunk with h1pre; fused per-partition sum
                    nc.scalar.activation(
                        out=ch, in_=h1_ps, func=AF.Copy,
                        accum_out=ssum[:, s:s + 1],
                    )
                    sqs = scr_pool.tile([EMB, N], F32, name="sqs", tag="sqs", bufs=2)
                    nc.vector.scalar_tensor_tensor(
                        out=sqs, in0=ch, scalar=0.0, in1=ch,
                        op0=ALU.add, op1=ALU.mult, accum_out=ssq[:, s:s + 1],
                    )

                # ---- summary branch (replicated; exact on every core) ----
                xs_tp = pstp.tile([128, KC, EMB], F32, name="xs_tp", tag="tp")
                for k in range(KC):
                    nc.tensor.transpose(
                        xs_tp[:, k, :], xmean[:, k * 128:(k + 1) * 128],
                        ident[0:EMB, 0:EMB],
                    )
                xsn = state.tile([128, KC, EMB], F32, name=f"xsn{l}", tag="xsn")
                nc.scalar.copy(xsn, xs_tp)
                aggs_ps = psa.tile([EMB, N], F32, name="aggs_ps", tag="aggps")
                for k in range(KC):
                    nc.tensor.matmul(
                        aggs_ps, xsn[:, k, :], aorig_sb[:, k, :],
                        start=(k == 0), stop=(k == KC - 1),
                    )
                aggs_sb = state.tile([EMB, N], F32, name=f"aggs_sb{l}", tag="aggssb")
                nc.scalar.copy(aggs_sb, aggs_ps)
                h2_ps = psb.tile([EMB, N], F32, name="h2_ps", tag="h1ps")
                nc.tensor.matmul(
                    h2_ps, wrels_sb[:, l, :], aggs_sb, start=True, stop=False
                )
                nc.tensor.matmul(
                    h2_ps, wroots_sb[:, l, :], xmean, start=False, stop=True
                )
                # local BN for the summary branch (N elems per channel)
                h2_sb = state.tile([EMB, N], F32, name=f"h2_sb{l}", tag="h2_sb")
                s2 = smol.tile([EMB, 1], F32, name=f"s2_{l}", tag="s2")
                nc.scalar.activation(
                    out=h2_sb, in_=h2_ps, func=AF.Copy, accum_out=s2
                )
                sq2 = scr_pool.tile([EMB, N], F32, name="sq2", tag="sqs", bufs=2)
                q2 = smol.tile([EMB, 1], F32, name=f"q2_{l}", tag="q2")
                nc.vector.scalar_tensor_tensor(
                    out=sq2, in0=h2_sb, scalar=0.0, in1=h2_sb,
                    op0=ALU.add, op1=ALU.mult, accum_out=q2,
                )
                mu2 = smol.tile([EMB, 1], F32, name=f"mu2_{l}", tag="mu2")
                nc.vector.tensor_scalar_mul(mu2, s2, 1.0 / N)
                m2sq = smol.tile([EMB, 1], F32, name=f"m2sq_{l}", tag="m2sq")
                nc.vector.tensor_tensor(out=m2sq, in0=mu2, in1=mu2, op=ALU.mult)
                var2 = smol.tile([EMB, 1], F32, name=f"var2_{l}", tag="var2")
                nc.vector.scalar_tensor_tensor(
                    out=var2, in0=q2, scalar=1.0 / N, in1=m2sq,
                    op0=ALU.mult, op1=ALU.subtract,
                )
                sg2, bp2 = bn_vectors(
                    mu2, var2, chv_sb[:, 2 * L + l:2 * L + l + 1],
                    chv_sb[:, 3 * L + l:3 * L + l + 1], f"s{l}",
                )
                h2t = state.tile([EMB, N], F32, name=f"h2t{l}", tag="h2t")
                nc.vector.tensor_scalar(
                    out=h2t, in0=h2_sb, scalar1=sg2, scalar2=bp2,
                    op0=ALU.mult, op1=ALU.add,
                )

                # ---- global BN stats for the main branch ----
                stat_in = smol.tile([EMB, 2], F32, name=f"stin{l}", tag="stin")
                nc.vector.reduce_sum(stat_in[:, 0:1], ssum, axis=AX.X)
                nc.vector.reduce_sum(stat_in[:, 1:2], ssq, axis=AX.X)
                if n_cores > 1:
                    sari = dram.tile([EMB, 2], F32, name=f"sari{l}", tag="sari")
                    saro = dram.tile(
                        [EMB, 2], F32, name=f"saro{l}", tag="saro",
                        addr_space=cc_space,
                    )
                    nc.sync.dma_start(out=sari, in_=stat_in)
                    nc.gpsimd.collective_compute(
                        "AllReduce", ALU.add, replica_groups=groups,
                        ins=[sari.opt()], outs=[saro.opt()],
                    )
                    stot = smol.tile([EMB, 2], F32, name=f"stot{l}", tag="stot")
                    nc.sync.dma_start(out=stot, in_=saro)
                else:
                    stot = stat_in
                mu = smol.tile([EMB, 1], F32, name=f"mu_{l}", tag="mu")
                nc.vector.tensor_scalar_mul(mu, stot[:, 0:1], 1.0 / nt_total)
                musq = smol.tile([EMB, 1], F32, name=f"musq_{l}", tag="musq")
                nc.vector.tensor_tensor(out=musq, in0=mu, in1=mu, op=ALU.mult)
                var = smol.tile([EMB, 1], F32, name=f"var_{l}", tag="var")
                nc.vector.scalar_tensor_tensor(
                    out=var, in0=stot[:, 1:2], scalar=1.0 / nt_total, in1=musq,
                    op0=ALU.mult, op1=ALU.subtract,
                )
                sg, bp = bn_vectors(
                    mu, var, chv_sb[:, l:l + 1], chv_sb[:, L + l:L + l + 1], f"m{l}"
                )
                addt = state.tile([EMB, N], F32, name=f"addt{l}", tag="addt")
                nc.vector.tensor_scalar_add(addt, h2t, bp)

                # ---- apply: h = relu(h1pre * sg + (h2t + bp)) ----
                for s in range(s_local):
                    ch = hbuf[:, s * N:(s + 1) * N]
                    ap_t = scr_pool.tile([EMB, N], F32, name="ap_t", tag="apt", bufs=3)
                    nc.vector.scalar_tensor_tensor(
                        out=ap_t, in0=ch, scalar=sg, in1=addt,
                        op0=ALU.mult, op1=ALU.add,
                    )
                    nc.scalar.activation(out=ch, in_=ap_t, func=AF.Relu)

            # ---- final: x_nodes -> log_softmax -> MLP ----
            xnm = xsum_allreduce("fin")  # [EMB, N] global mean over subgraphs
            xn_tp = pstp.tile([128, KC, EMB], F32, name="xn_tp", tag="tp")
            for k in range(KC):
                nc.tensor.transpose(
                    xn_tp[:, k, :], xnm[:, k * 128:(k + 1) * 128],
                    ident[0:EMB, 0:EMB],
                )
            xn = state.tile([128, KC, EMB], F32, name="xn", tag="xn")
            nc.scalar.copy(xn, xn_tp)
            mx = smol.tile([128, KC], F32, name="mx", tag="mx")
            nc.vector.reduce_max(mx, xn, axis=AX.X)
            nmx = smol.tile([128, KC], F32, name="nmx", tag="nmx")
            nc.vector.tensor_scalar_mul(nmx, mx, -1.0)
            ex = state.tile([128, KC, EMB], F32, name="ex", tag="ex")
            se = smol.tile([128, KC], F32, name="se", tag="se")
            for k in range(KC):
                nc.scalar.activation(
                    out=ex[:, k, :], in_=xn[:, k, :], func=AF.Exp,
                    bias=nmx[:, k:k + 1], accum_out=se[:, k:k + 1],
                )
            lnse = smol.tile([128, KC], F32, name="lnse", tag="lnse")
            nc.scalar.activation(out=lnse, in_=se, func=AF.Ln)
            zt = state.tile([128, KC, EMB], F32, name="zt", tag="zt")
            for k in range(KC):
                nc.vector.tensor_scalar(
                    out=zt[:, k, :], in0=xn[:, k, :], scalar1=nmx[:, k:k + 1],
                    scalar2=lnse[:, k:k + 1], op0=ALU.add, op1=ALU.subtract,
                )
            # transpose z back to [EMB, N]
            zT_ps = psb.tile([EMB, KC, 128], F32, name="zT_ps", tag="h1ps")
            for k in range(KC):
                nc.tensor.transpose(zT_ps[:, k, :], zt[:, k, :], ident)
            zT = state.tile([EMB, N], F32, name="zT", tag="zT")
            nc.scalar.copy(zT, zT_ps)
            # MLP (transposed): m = relu(W1 @ zT + b1); o = W2 @ m + b2
            m_ps = psa.tile([2 * EMB, N], F32, name="m_ps", tag="aggps")
            nc.tensor.matmul(m_ps, w1_sb, zT, start=True, stop=True)
            m_sb = state.tile([2 * EMB, N], F32, name="m_sb", tag="m_sb")
            nc.scalar.activation(out=m_sb, in_=m_ps, func=AF.Relu, bias=b1_sb)
            o_ps = psb.tile([TASKS, N], F32, name="o_ps", tag="h1ps")
            nc.tensor.matmul(o_ps, w2_sb, m_sb, start=True, stop=True)
            o_sb = state.tile([TASKS, N], F32, name="o_sb", tag="o_sb")
            nc.scalar.activation(out=o_sb, in_=o_ps, func=AF.Identity, bias=b2_sb)
            nc.sync.dma_start(out=outT, in_=o_sb)

    nc.compile()
    return nc


def prep_in_maps(inputs, n_cores=N_CORES, s_local=S_LOCAL, agg_bf16=True):
    """Host-side sharding/densification. Returns list of per-core in_maps."""
    nodes = s_local * N
    adt = BF if agg_bf16 else np.float32
    g = {k: np.asarray(v) for k, v in inputs.items()}
    x = g["x"].astype(np.float32)
    ei = g["edge_index"].astype(np.int64)
    oe = g["original_edge_index"].astype(np.int64)

    assert int(g["num_nodes_int"]) == N
    assert x.shape == (n_cores * nodes, EMB)
    sni = np.asarray(g["subgraph_node_idx"])
    assert (sni == np.tile(np.arange(N, dtype=sni.dtype), n_cores * s_local)).all(), \
        "kernel assumes subgraph_node_idx == tile(arange(N), S)"
    eg = ei[0] // N
    assert (eg == ei[1] // N).all(), "edges must stay within a subgraph"

    src_l = ei[0] % N
    dst_l = ei[1] % N

    a_orig = np.bincount(oe[0] * N + oe[1], minlength=N * N) \
        .reshape(N, N).astype(np.float32)

    wrelT = np.ascontiguousarray(g["Wrel"].transpose(2, 0, 1)).astype(np.float32)
    wrootT = np.ascontiguousarray(g["Wroot"].transpose(2, 0, 1)).astype(np.float32)
    wrelsT = np.ascontiguousarray(g["Wrel_s"].transpose(2, 0, 1)).astype(np.float32)
    wrootsT = np.ascontiguousarray(g["Wroot_s"].transpose(2, 0, 1)).astype(np.float32)
    chvecs = np.concatenate(
        [g["bn_gamma"].T, g["bn_beta"].T, g["bns_gamma"].T, g["bns_beta"].T],
        axis=1,
    ).astype(np.float32)  # [EMB, 4L]
    w1T = np.ascontiguousarray(g["W1"].T).astype(np.float32)
    b1c = g["b1"].reshape(2 * EMB, 1).astype(np.float32)
    w2T = np.ascontiguousarray(g["W2"].T).astype(np.float32)
    b2c = g["b2"].reshape(TASKS, 1).astype(np.float32)

    in_maps = []
    for c in range(n_cores):
        lo, hi = c * s_local, (c + 1) * s_local
        m = (eg >= lo) & (eg < hi)
        ids = ((eg[m] - lo) * N + src_l[m]) * N + dst_l[m]
        a_big = np.bincount(ids, minlength=s_local * N * N) \
            .reshape(s_local, N, N).astype(adt)
        h0T = np.ascontiguousarray(x[c * nodes:(c + 1) * nodes].T)
        in_maps.append(dict(
            a_big=a_big, h0T=h0T, a_orig=a_orig,
            wrelT=wrelT, wrootT=wrootT, wrelsT=wrelsT, wrootsT=wrootsT,
            chvecs=chvecs, w1T=w1T, b1c=b1c, w2T=w2T, b2c=b2c,
        ))
    return in_maps


_NC_CACHE = {}


def kernel(**inputs) -> np.ndarray:
    key = (N_CORES, S_LOCAL, True)
    if key not in _NC_CACHE:
        _NC_CACHE[key] = build_nc(*key)
    nc = _NC_CACHE[key]
    in_maps = prep_in_maps(inputs, N_CORES, S_LOCAL, agg_bf16=True)
    res = run_bass_kernel_spmd(nc, in_maps, core_ids=list(range(N_CORES)))
    out = res.results[0]["outT"]  # [TASKS, N]
    return np.ascontiguousarray(out.T).astype(np.float32)
